# revision 4
# baseline (speedup 1.0000x reference)
"""Trainium2 Bass kernel for nn_SamplePolicy_14886356648064.

Reference semantics (T=4 resample rounds, K=4 vote threshold, H=8 heads):
  each round: per-head argmax over src -> presence vector per head ->
  counting = sum of presence over heads -> trigger = counting.max() <= K ->
  if trigger, replace all heads with head `sampled_t` (broadcast).

Exact algebraic collapse of the T-loop:
  - If trigger is False for the initial aw, the state never changes, so the
    trigger value is identical every round -> output == input.
  - If trigger fires (round 0), all heads become identical; afterwards each
    src position in the (common) argmax set receives H=8 votes > K=4, so the
    trigger can never fire again -> output == broadcast(aw[sampled_0]).
  Hence only round 0's trigger and sampled_0 matter, for ANY input.

Device work: the input is cast on the host to fp16 (a monotone, pointwise
rounding -- halves HBM bytes), sharded one head per NeuronCore.  Each core
streams its 16.8MB fp16 head slice once and reduces every row window to 128
"cell" maxima, where cell c of a window holds positions {c + 128*k}.  The
reduction is an in-place contiguous halving tree on the vector engine
(tensor_tensor max runs at 2 elem/cycle/lane for 16-bit dtypes, vs 1x for
tensor_reduce), with fold ops batched over groups of row tiles to amortize
the ~200ns/op + ~27ns/segment instruction overhead.  The host then gathers
only the candidate cells (those attaining the row's fp16 max -- a monotone
map guarantees the true argmax's cell is among them), resolves the exact
first-occurrence f32 argmax, and runs the tiny vote/trigger logic.

sampled_0 = jax.random.randint(jax.random.fold_in(jax.random.key(42), 0),
                               (), 0, 7) == 3 (threefry, platform independent).
"""

import numpy as np

H = 8
TGT = 2048
SRC = 4096
P = 128            # SBUF partitions per tile
NTILES = TGT // P  # 16 row tiles per head
C = 128            # cells per chunk (residues mod 128 of the chunk window)
K_THRESH = 4
SAMPLED_T0 = 3

# Fold work units: groups of full row-tiles, then the last tile split into
# two column halves so the final folds start before the whole stream lands.
# The first tiles load singly so the vector engine starts folding as early
# as possible; later tiles load in pairs (bigger DMAs, fewer fold ops).
# groups: (first_tile, n_tiles) over tiles 0..14; tile 15 is the two halves.
GROUPS = [(0, 1), (1, 1), (2, 1), (3, 1), (4, 2), (6, 2), (8, 2), (10, 2),
          (12, 2), (14, 1)]
NCHUNK = NTILES + 1  # 15 full-tile chunks + 2 half chunks of tile 15
N_STORE_A = 10       # chunk slots flushed early (overlapped with tail loads)

_cache = {}


def _build_nc():
    """Raw Bass program, one head per core, fp16 input.

    All 16 row-tiles are SBUF-resident (128KB/partition).  Pair-loads (2MB)
    alternate between the two HWDGE rings; the vector engine runs an in-place
    halving max-tree per group as soon as that group's load lands.  Cells of a
    4096-wide row window end up in the window's first 128 columns.
    """
    from contextlib import ExitStack

    import concourse.bass as bass
    import concourse.mybir as mybir

    nc = bass.Bass()
    f16 = mybir.dt.float16
    x = nc.declare_dram_parameter("x", [TGT, SRC], f16, isOutput=False)
    bm = nc.declare_dram_parameter("bm", [P, NCHUNK, C], f16, isOutput=True)

    with ExitStack() as ctx:
        tiles = ctx.enter_context(nc.sbuf_tensor([P, NTILES, SRC], f16))
        bmsb = ctx.enter_context(nc.sbuf_tensor([P, NCHUNK, C], f16))
        # one completion semaphore per load group
        n_loads = len(GROUPS) + 2
        s_g = [ctx.enter_context(nc.semaphore(f"s_g{j}")) for j in range(n_loads)]
        s_da = ctx.enter_context(nc.semaphore("s_da"))
        s_db = ctx.enter_context(nc.semaphore("s_db"))
        s_out = ctx.enter_context(nc.semaphore("s_out"))
        block = ctx.enter_context(nc.Block())

        # DRAM view [p, tile, col] of the row-major [2048, 4096] head slice
        xv = x[:, :].rearrange("(t p) c -> p t c", p=P)

        def issue_loads(eng, parity):
            for j, (t0, nt) in enumerate(GROUPS):
                if j % 2 != parity:
                    continue
                eng.dma_start(
                    out=tiles[:, t0 : t0 + nt, :],
                    in_=xv[:, t0 : t0 + nt, :],
                ).then_inc(s_g[j], 16)
            # tile 15 halves (tapered tail)
            for j, c0 in enumerate((0, SRC // 2)):
                if (len(GROUPS) + j) % 2 != parity:
                    continue
                eng.dma_start(
                    out=tiles[:, NTILES - 1, c0 : c0 + SRC // 2],
                    in_=xv[:, NTILES - 1, c0 : c0 + SRC // 2],
                ).then_inc(s_g[len(GROUPS) + j], 16)

        @block.sync
        def _(sync):
            issue_loads(sync, 0)
            sync.wait_ge(s_out, 32)

        @block.scalar
        def _(scalar):
            issue_loads(scalar, 1)

        @block.gpsimd
        def _(gpsimd):
            # bm stores ride the (otherwise idle) SWDGE queue so the early
            # flush overlaps the HWDGE load stream and tail folds.
            gpsimd.wait_ge(s_da, 1)
            gpsimd.dma_start(
                out=bm[:, :N_STORE_A, :], in_=bmsb[:, :N_STORE_A, :]
            ).then_inc(s_out, 16)
            gpsimd.wait_ge(s_db, 1)
            gpsimd.dma_start(
                out=bm[:, N_STORE_A:, :], in_=bmsb[:, N_STORE_A:, :]
            ).then_inc(s_out, 16)

        @block.vector
        def _(vector):
            mx = mybir.AluOpType.max

            def fold(t0, nt, c0, w0):
                # halving max-tree over window [c0, c0+2*w0) of tiles t0..t0+nt
                w = w0
                while w >= C:
                    nc.vector.tensor_tensor(
                        out=tiles[:, t0 : t0 + nt, c0 : c0 + w],
                        in0=tiles[:, t0 : t0 + nt, c0 : c0 + w],
                        in1=tiles[:, t0 : t0 + nt, c0 + w : c0 + 2 * w],
                        op=mx,
                    )
                    w //= 2

            for j, (t0, nt) in enumerate(GROUPS):
                vector.wait_ge(s_g[j], 16)
                fold(t0, nt, 0, SRC // 2)
                if t0 + nt == N_STORE_A:
                    # early flush of the first chunk slots
                    nc.vector.tensor_copy(
                        out=bmsb[:, :N_STORE_A, :],
                        in_=tiles[:, :N_STORE_A, 0:C],
                    ).then_inc(s_da, 1)
            for j, c0 in enumerate((0, SRC // 2)):
                vector.wait_ge(s_g[len(GROUPS) + j], 16)
                fold(NTILES - 1, 1, c0, SRC // 4)
            # remaining cell maxima: chunk slots 10..14 = tiles 10..14,
            # slot 15 = tile 15 cols 0:2048 folded, slot 16 = tile 15
            # second-half fold at cols 2048:2176.
            nc.vector.tensor_copy(
                out=bmsb[:, N_STORE_A:NTILES, :],
                in_=tiles[:, N_STORE_A:, 0:C],
            )
            nc.vector.tensor_copy(
                out=bmsb[:, NTILES, :],
                in_=tiles[:, NTILES - 1, SRC // 2 : SRC // 2 + C],
            ).then_inc(s_db, 1)

    return nc


def _get_nc():
    if "nc" not in _cache:
        _cache["nc"] = _build_nc()
    return _cache["nc"]


def run_device(aw16, **run_kwargs):
    """Run the per-head cell-max kernel on 8 cores.

    aw16: [H, TGT, SRC] float16. Returns ([H, P, NCHUNK, C] float16, results).
    """
    from concourse.bass_utils import run_bass_kernel_spmd

    nc = _get_nc()
    in_maps = [{"x": np.ascontiguousarray(aw16[c])} for c in range(H)]
    res = run_bass_kernel_spmd(nc, in_maps, list(range(H)), **run_kwargs)
    bm = np.stack([res.results[c]["bm"] for c in range(H)])
    return bm, res


def _host_cellmax(aw16):
    """Numpy fallback producing the same [H, P, NCHUNK, C] cell maxima."""
    bm = np.empty((H, P, NCHUNK, C), np.float16)
    full = aw16.reshape(H, NTILES, P, SRC // C, C).max(axis=3)  # [H,16,P,C]
    bm[:, :, :NTILES, :] = full.transpose(0, 2, 1, 3)
    tail = aw16[:, (NTILES - 1) * P :, :].reshape(H, P, 2, SRC // C // 2, C)
    bm[:, :, NTILES - 1, :] = tail[:, :, 0].max(axis=2)
    bm[:, :, NTILES, :] = tail[:, :, 1].max(axis=2)
    return bm


def _exact_argmax(aw, bm):
    """Exact first-occurrence np.argmax(aw, -1) from device cell maxima.

    aw: [H, TGT, SRC] float32; bm: [H, P, NCHUNK, C] float16 where slot
    t<15 covers rows t*128+p over the full 4096 cols, slots 15/16 cover
    tile-15 rows over column windows [0,2048) and [2048,4096).
    """
    BIG = 1 << 20
    ntail = (NTILES - 1) * P  # 1920 rows with a single full-row chunk

    # full-row chunks -> [H, 1920, C]
    full_bm = bm[:, :, : NTILES - 1, :].transpose(0, 2, 1, 3).reshape(H, ntail, C)
    tailA = bm[:, :, NTILES - 1, :]  # [H, P, C], window [0, 2048)
    tailB = bm[:, :, NTILES, :]      # [H, P, C], window [2048, 4096)

    rowmax_full = full_bm.max(-1)
    rowmax_tail = np.maximum(tailA.max(-1), tailB.max(-1))

    cand_m = []
    cand_pos = []
    cand_row = []

    # type A: rows 0..1919, cell c covers positions c + 128k, k < 32
    ha, ra, ca = np.nonzero(full_bm == rowmax_full[..., None])
    if ha.size:
        awr = aw[:, :ntail].reshape(H, ntail, SRC // C, C)
        vals = awr[ha, ra, :, ca]  # [NA, 32]
        m = vals.max(1)
        k = np.where(vals == m[:, None], np.arange(SRC // C), BIG).min(1)
        cand_m.append(m)
        cand_pos.append(ca + C * k)
        cand_row.append(ha * TGT + ra)

    # type B: tile-15 rows, two half windows, cell depth 16
    awt = aw[:, ntail:].reshape(H, P, 2, SRC // C // 2, C)
    for wi, tbm in enumerate((tailA, tailB)):
        hb, pb, cb = np.nonzero(tbm == rowmax_tail[..., None])
        if hb.size:
            vals = awt[hb, pb, wi, :, cb]  # [NB, 16]
            m = vals.max(1)
            k = np.where(vals == m[:, None], np.arange(SRC // C // 2), BIG).min(1)
            cand_m.append(m)
            cand_pos.append(wi * (SRC // 2) + cb + C * k)
            cand_row.append(hb * TGT + (ntail + pb))

    m = np.concatenate(cand_m)
    pos = np.concatenate(cand_pos)
    row = np.concatenate(cand_row)
    order = np.argsort(row, kind="stable")
    m, pos, row = m[order], pos[order], row[order]
    starts = np.flatnonzero(np.r_[True, row[1:] != row[:-1]])
    urow = row[starts]
    assert urow.size == H * TGT, f"missing rows: {urow.size}"
    best = np.maximum.reduceat(m, starts)
    seg = np.repeat(np.arange(starts.size), np.diff(np.r_[starts, row.size]))
    bestpos = np.minimum.reduceat(
        np.where(m == best[seg], pos, 1 << 30), starts
    )
    out = np.empty(H * TGT, np.int64)
    out[urow] = bestpos
    return out.reshape(H, TGT)


def kernel(attention_weight):
    aw = np.asarray(attention_weight)
    assert aw.shape == (H, TGT, SRC), aw.shape
    aw = aw.astype(np.float32, copy=False)
    aw16 = aw.astype(np.float16)

    try:
        bm, _ = run_device(aw16)
    except Exception as e:  # device path failed: fall back to host cellmax
        import traceback

        traceback.print_exc()
        print(f"WARNING: device path failed ({e!r}); falling back to numpy")
        bm = _host_cellmax(aw16)

    cand = _exact_argmax(aw, bm)  # [H, TGT]
    present = np.zeros((H, SRC), np.float32)
    present[np.arange(H)[:, None], cand] = 1.0
    counting = present.sum(axis=0)

    if counting.max() <= K_THRESH:
        return np.broadcast_to(aw[SAMPLED_T0], aw.shape).copy()
    return aw
